# revision 36
# baseline (speedup 1.0000x reference)
"""Trainium2 Bass kernel for a 2-layer GATv2 (DependencyGraphAnalyzer).

Strategy (8 cores, SPMD, edge-parallel by dst range):
  - Host sorts edges by dst and shards them by dst-node range: core c owns
    nodes [c*2500, (c+1)*2500) and every edge pointing into that range, so
    softmax segments are core-local.
  - All compute in bf16 (fp32 PSUM accumulation); one-hot scatter/gather
    matrices are precomputed on the host in fp8 and kept SBUF-resident.
  - Dense phase: every core computes xl = x@Wl+bl for ALL nodes (replicated,
    cheap in bf16) and writes it to a DRAM table in bf16; xr for its own
    2500 nodes stays in SBUF.
  - Edge phase per 128-dst-node block, in chunks of GCH 128-edge tiles:
    dma_gather of xl[src] rows (bf16, 4 SWDGE queues round-robin), one-hot
    matmul for xr[dst], then chunk-batched vector ops:
      v = xg + pxr; e = max(.2v, v); t2 = e*att; sc = reduce_c(t2);
      ex = exp(sc); msg = xg * ex
    and per-tile scatter matmuls pout += oh@msg, pden += oh@ex.
    Scatter matmuls run one chunk behind (software pipeline) so the
    in-order tensor queue never waits on the vector chain.
  - Softmax max-subtraction is skipped (scores are O(1); fp32 exp is safe).
  - Two launches; host all-gathers the bf16 hidden state h between layers.
"""

import numpy as np
import ml_dtypes

# Problem constants (hardcoded; kernel.py must be self-contained).
N_NODES = 20000
N_EDGES = 320000
IN_DIM = 256
HID = 128
HEADS = 4
NEG_SLOPE = 0.2
NCORES = 8
NPC = N_NODES // NCORES  # 2500 own nodes per core
P = 128

BF16 = ml_dtypes.bfloat16
FP8 = ml_dtypes.float8_e4m3


# ---------------------------------------------------------------------------
# Host-side edge preprocessing
# ---------------------------------------------------------------------------

def prep_edges(edge_index, n_nodes=N_NODES, ncores=NCORES):
    """Sort edges by dst, shard by dst range, pad each (core, block) segment
    to a common per-block tile count, and build the device index arrays.

    Returns (Tb, per_core): Tb[b] = number of 128-edge tiles of block b
    (shared by all cores); per_core[c] holds:
      idx : [128, TT*8] int16  wrapped dma_gather indices (pad -> n_nodes)
      oh  : [128, TT*128] fp8  oh[p, t*128+j]  = (dst_local[t*128+p] == j)
      ohT : [128, TT*128] fp8  ohT[j, t*128+p] = (dst_local[t*128+p] == j)
    """
    npc = n_nodes // ncores
    nb = (npc + P - 1) // P
    src = np.asarray(edge_index[0], dtype=np.int64)
    dst = np.asarray(edge_index[1], dtype=np.int64)
    order = np.argsort(dst, kind="stable")
    src_s, dst_s = src[order], dst[order]

    core_of = dst_s // npc
    blk_of = core_of * nb + (dst_s - core_of * npc) // P
    counts = np.bincount(blk_of, minlength=ncores * nb).reshape(ncores, nb)
    ends = np.cumsum(counts.reshape(-1)).reshape(ncores, nb)
    starts = ends - counts

    tiles = (counts + P - 1) // P
    Tb = np.maximum(tiles.max(axis=0), 1)
    TT = int(Tb.sum())
    offs = np.concatenate([[0], np.cumsum(Tb)[:-1]])

    jj = np.arange(P, dtype=np.int32)
    per_core = []
    for c in range(ncores):
        idx_flat = np.full(TT * P, n_nodes, dtype=np.int64)  # pad -> zero row
        dloc_flat = np.full(TT * P, -1, dtype=np.int32)      # pad -> -1
        for b in range(nb):
            s, e = starts[c, b], ends[c, b]
            cnt = e - s
            o = offs[b] * P
            idx_flat[o:o + cnt] = src_s[s:e]
            dloc_flat[o:o + cnt] = (dst_s[s:e] - c * npc - b * P).astype(np.int32)
        # Wrap gather indices per block: within a gather call of n idxs,
        # index j lives at [j % 16, j // 16]; replicate rows to 128 parts.
        idx_w = np.zeros((P, TT * 8), dtype=np.int16)
        for b in range(nb):
            o = offs[b] * P
            n = int(Tb[b]) * P
            seg = idx_flat[o:o + n].astype(np.int16)
            w = seg.reshape(n // 16, 16).T  # [16, n/16]
            idx_w[:, o // 16:(o + n) // 16] = np.tile(w, (8, 1))
        # One-hot tables, fp8. E[t, p, j] = (dloc[t*128+p] == j)
        dl = dloc_flat.reshape(TT, P)
        E = (dl[:, :, None] == jj[None, None, :])
        oh = np.ascontiguousarray(
            E.transpose(1, 0, 2).reshape(P, TT * P)).astype(FP8)
        ohT = np.ascontiguousarray(
            E.transpose(2, 0, 1).reshape(P, TT * P)).astype(FP8)
        per_core.append({"idx": idx_w, "oh": oh, "ohT": ohT})
    return [int(t) for t in Tb], per_core


# ---------------------------------------------------------------------------
# Bass program builder (one GATv2 layer, optionally + heads)
# ---------------------------------------------------------------------------

def build_layer(nn, npc, in_dim, heads, Tb, elu, heads_out, gch,
                num_devices=NCORES):
    """Build + compile one layer program. Returns the Bacc object."""
    import concourse.bacc as bacc
    import concourse.tile as tile
    import concourse.mybir as mybir
    from contextlib import ExitStack

    f32 = mybir.dt.float32
    bf16 = mybir.dt.bfloat16
    fp8 = mybir.dt.float8e4
    i16 = mybir.dt.int16
    AF = mybir.ActivationFunctionType
    ALU = mybir.AluOpType
    AX = mybir.AxisListType

    C = HID
    H = heads
    F = H * C
    KC = in_dim // P
    NB = len(Tb)
    TT = sum(Tb)
    blk_rows = [min(P, npc - b * P) for b in range(NB)]
    n_node_tiles = (nn + P - 1) // P
    # L2 folds the denominator into pout as an extra ones-column (F+1 cols).
    import os
    fuse_den = (H == 1) and os.environ.get("NOFUSE", "") != "1"
    FO = F + 1 if fuse_den else F

    nc = bacc.Bacc("TRN2", target_bir_lowering=False, debug=False,
                   num_devices=num_devices, num_swdge_queues=4)

    # --- DRAM tensors ---
    xT = nc.dram_tensor("xT", [in_dim, nn], bf16, kind="ExternalInput").ap()
    xoT = nc.dram_tensor("xoT", [in_dim, npc], bf16, kind="ExternalInput").ap()
    wl = nc.dram_tensor("wl", [in_dim, F], bf16, kind="ExternalInput").ap()
    wr = nc.dram_tensor("wr", [in_dim, F], bf16, kind="ExternalInput").ap()
    bl = nc.dram_tensor("bl", [1, F], bf16, kind="ExternalInput").ap()
    br = nc.dram_tensor("br", [1, F], bf16, kind="ExternalInput").ap()
    attb = nc.dram_tensor("attb", [P, gch * F], bf16, kind="ExternalInput").ap()
    biasb = nc.dram_tensor("biasb", [P, F], bf16, kind="ExternalInput").ap()
    ones_d = nc.dram_tensor("ones", [1, P], bf16, kind="ExternalInput").ap()
    idx_d = nc.dram_tensor("idx", [P, TT * 8], i16, kind="ExternalInput").ap()
    oh_d = nc.dram_tensor("oh", [P, TT * P], fp8, kind="ExternalInput").ap()
    ohT_d = nc.dram_tensor("ohT", [P, TT * P], fp8, kind="ExternalInput").ap()
    if heads_out:
        headw = nc.dram_tensor("headw", [P, 2 * C], bf16,
                               kind="ExternalInput").ap()
        headb = nc.dram_tensor("headb", [P, 2], f32, kind="ExternalInput").ap()
        an_d = nc.dram_tensor("an", [npc, 1], f32, kind="ExternalOutput").ap()
        rc_d = nc.dram_tensor("rc", [npc, 1], f32, kind="ExternalOutput").ap()
    else:
        h_d = nc.dram_tensor("h_own", [npc, F], bf16, kind="ExternalOutput").ap()

    xl_full = nc.dram_tensor("xl_full", [nn + 1, F], bf16, kind="Internal").ap()

    with tile.TileContext(nc) as tc, ExitStack() as es:
        cp = es.enter_context(tc.tile_pool(name="const", bufs=1))

        wl_sb = cp.tile([P, KC, F], bf16, tag="wl")
        nc.sync.dma_start(wl_sb[:], wl.rearrange("(c k) f -> k c f", k=P))
        wr_sb = cp.tile([P, KC, F], bf16, tag="wr")
        nc.sync.dma_start(wr_sb[:], wr.rearrange("(c k) f -> k c f", k=P))
        bl_sb = cp.tile([1, F], bf16, tag="bl")
        nc.sync.dma_start(bl_sb[:], bl[:])
        br_sb = cp.tile([1, F], bf16, tag="br")
        nc.sync.dma_start(br_sb[:], br[:])
        attb_sb = cp.tile([P, gch, H, C], bf16, tag="attb")
        nc.sync.dma_start(attb_sb[:],
                          attb.rearrange("p (g h c) -> p g h c", g=gch, h=H))
        biasb_sb = cp.tile([P, H, C], bf16, tag="biasb")
        nc.sync.dma_start(biasb_sb[:], biasb.rearrange("p (h c) -> p h c", h=H))
        ones_sb = cp.tile([1, P], bf16, tag="ones")
        nc.sync.dma_start(ones_sb[:], ones_d[:])
        idx_sb = cp.tile([P, TT * 8], i16, tag="idx")
        nc.sync.dma_start(idx_sb[:], idx_d[:])
        oh_sb = cp.tile([P, TT, P], fp8, tag="oh")
        nc.sync.dma_start(oh_sb[:], oh_d.rearrange("p (t j) -> p t j", j=P))
        ohT_sb = cp.tile([P, TT, P], fp8, tag="ohT")
        nc.sync.dma_start(ohT_sb[:], ohT_d.rearrange("p (t j) -> p t j", j=P))
        if heads_out:
            headw_sb = cp.tile([P, 2 * C], bf16, tag="headw")
            nc.sync.dma_start(headw_sb[:], headw[:])
            headb_sb = cp.tile([P, 2], f32, tag="headb")
            nc.sync.dma_start(headb_sb[:], headb[:])

        xr_sb = cp.tile([P, NB, F], bf16, tag="xr")
        nc.vector.memset(xr_sb[:], 0.0)

        # ---- dense phase: xl_full = x @ wl + bl (all nodes), xr (own) ----
        # MT node tiles share one DMA load/store (HWDGE sequencer is the
        # dense-phase bottleneck at ~0.7us per dma_start).
        MT = 4
        with tc.tile_pool(name="dpsum", bufs=2, space="PSUM") as dps, \
             tc.tile_pool(name="dwork", bufs=3) as dw:
            for m0 in range(0, n_node_tiles, MT):
                mte = min(m0 + MT, n_node_tiles)
                rr = min(MT * P, nn - m0 * P)
                lt = dw.tile([P, KC, MT * P], bf16, tag="lt")
                nc.sync.dma_start(
                    lt[:, :, :rr],
                    xT[:, m0 * P:m0 * P + rr].rearrange("(c k) n -> k c n", k=P))
                ot = dw.tile([P, MT, F], bf16, tag="ot")
                for m in range(m0, mte):
                    r = min(P, nn - m * P)
                    t = m - m0
                    ps = dps.tile([P, F], f32, tag="ps")
                    for c in range(KC):
                        nc.tensor.matmul(
                            ps[:r], lhsT=lt[:, c, t * P:t * P + r],
                            rhs=wl_sb[:, c, :], start=(c == 0), stop=False)
                    nc.tensor.matmul(ps[:r], lhsT=ones_sb[:, :r], rhs=bl_sb[:],
                                     start=False, stop=True)
                    nc.scalar.copy(ot[:r, t, :], ps[:r])
                if rr == (mte - m0) * P:
                    nc.sync.dma_start(
                        xl_full[m0 * P:m0 * P + rr, :].rearrange(
                            "(t p) f -> p t f", p=P),
                        ot[:, :mte - m0, :])
                else:
                    for m in range(m0, mte):
                        r = min(P, nn - m * P)
                        nc.sync.dma_start(xl_full[m * P:m * P + r, :],
                                          ot[:r, m - m0, :])
            zt = dw.tile([1, F], bf16, tag="zt")
            nc.vector.memset(zt[:], 0.0)
            nc.sync.dma_start(xl_full[nn:nn + 1, :], zt[:])

            for b0 in range(0, NB, MT):
                bte = min(b0 + MT, NB)
                rr = min(MT * P, npc - b0 * P)
                lt = dw.tile([P, KC, MT * P], bf16, tag="lt")
                nc.sync.dma_start(
                    lt[:, :, :rr],
                    xoT[:, b0 * P:b0 * P + rr].rearrange("(c k) n -> k c n",
                                                         k=P))
                for b in range(b0, bte):
                    r = blk_rows[b]
                    t = b - b0
                    ps = dps.tile([P, F], f32, tag="ps")
                    for c in range(KC):
                        nc.tensor.matmul(
                            ps[:r], lhsT=lt[:, c, t * P:t * P + r],
                            rhs=wr_sb[:, c, :], start=(c == 0), stop=False)
                    nc.tensor.matmul(ps[:r], lhsT=ones_sb[:, :r], rhs=br_sb[:],
                                     start=False, stop=True)
                    nc.vector.tensor_copy(xr_sb[:r, b, :], ps[:r])

        tc.strict_bb_all_engine_barrier()

        # ---- edge phase ----
        # compute chunks: (block, tile0, ntiles, global tile offset, first,
        #                  last, gather chunk idx, offset within gather chunk)
        # gather chunks: (global tile offset, ntiles) of up to GG tiles
        GG = 9
        chunks = []
        gchunks = []
        off = 0
        for b in range(NB):
            for g0 in range(0, Tb[b], GG):
                gn = min(GG, Tb[b] - g0)
                gi = len(gchunks)
                gchunks.append((off + g0, gn))
                for t0 in range(g0, g0 + gn, gch):
                    n_t = min(gch, g0 + gn - t0)
                    chunks.append((b, t0, n_t, off + t0,
                                   t0 == 0, t0 + n_t == Tb[b], gi, t0 - g0))
            off += Tb[b]
        nch = len(chunks)

        with tc.tile_pool(name="gxg", bufs=3) as gxg, \
             tc.tile_pool(name="ew", bufs=3) as ew, \
             tc.tile_pool(name="ep", bufs=2) as epp, \
             tc.tile_pool(name="ps_xr", bufs=2, space="PSUM") as ps_xr_p, \
             tc.tile_pool(name="ps_out", bufs=1, space="PSUM") as ps_out_p, \
             tc.tile_pool(name="ps_den", bufs=1, space="PSUM") as ps_den_p:

            xg_t = [None] * len(gchunks)
            msg_t = [None] * nch
            ex_t = [None] * nch
            xgr_t = [None] * nch
            pout = pden = None

            def epilogue(b):
                r = blk_rows[b]
                if fuse_den:
                    den = epp.tile([P, 1], f32, tag="den")
                    nc.vector.tensor_scalar_add(den[:], pout[:, F:F + 1], 1e-16)
                    rec = epp.tile([P, 1, 1], f32, tag="rec")
                    nc.vector.reciprocal(rec[:, 0, :], den[:])
                    recb = rec[:].to_broadcast([P, H, C])
                else:
                    den = epp.tile([P, H, 1], f32, tag="den")
                    nc.vector.tensor_scalar_add(den[:, :, 0], pden[:], 1e-16)
                    rec = epp.tile([P, H, 1], f32, tag="rec")
                    nc.vector.reciprocal(rec[:], den[:])
                    recb = rec[:].to_broadcast([P, H, C])
                hb = epp.tile([P, H, C], bf16, tag="hb")
                nc.vector.tensor_tensor(
                    out=hb[:], in0=pout[:, :F].rearrange("p (h c) -> p h c", c=C),
                    in1=recb, op=ALU.mult)
                hc = epp.tile([P, H, C], bf16, tag="hc")
                nc.vector.tensor_tensor(out=hc[:], in0=hb[:], in1=biasb_sb[:],
                                        op=ALU.add)
                if elu:
                    mn = epp.tile([P, H, C], bf16, tag="mn")
                    nc.vector.tensor_scalar_min(mn[:], hc[:], 0.0)
                    en = epp.tile([P, H, C], f32, tag="en")
                    nc.scalar.activation(en[:], mn[:], AF.Exp)
                    mx = epp.tile([P, H, C], bf16, tag="mx")
                    nc.vector.tensor_scalar_max(mx[:], hc[:], 0.0)
                    ho = epp.tile([P, H, C], bf16, tag="ho")
                    nc.vector.scalar_tensor_tensor(
                        out=ho[:], in0=mx[:], scalar=-1.0, in1=en[:],
                        op0=ALU.add, op1=ALU.add)
                    nc.sync.dma_start(
                        h_d[b * P:b * P + r, :],
                        ho[:r].rearrange("p h c -> p (h c)"))
                elif heads_out:
                    # logits only; sigmoid is applied on the host
                    for j, outd in enumerate([an_d, rc_d]):
                        scr = epp.tile([P, C], bf16, tag="scr")
                        nc.vector.tensor_tensor(
                            out=scr[:], in0=hc[:, 0, :],
                            in1=headw_sb[:, j * C:(j + 1) * C], op=ALU.mult)
                        red = epp.tile([P, 1], f32, tag="red")
                        nc.vector.tensor_reduce(out=red[:], in_=scr[:],
                                                axis=AX.X, op=ALU.add)
                        nc.sync.dma_start(outd[b * P:b * P + r, :], red[:r])
                else:
                    nc.sync.dma_start(
                        h_d[b * P:b * P + r, :],
                        hc[:r].rearrange("p h c -> p (h c)"))

            for k in range(nch + 1):
                if k < nch:
                    b, t0, n_t, g0, first, last, gi, go = chunks[k]
                    # 1. gather xl[src] rows, GG tiles per call (4 queues, rr)
                    if xg_t[gi] is None:
                        gg0, gn = gchunks[gi]
                        xg = gxg.tile([P, GG, H, C], bf16, tag="xg")
                        nc.gpsimd.dma_gather(
                            xg[:, :gn, :, :].rearrange("p g h c -> p g (h c)"),
                            xl_full[:], idx_sb[:, gg0 * 8:(gg0 + gn) * 8],
                            num_idxs=gn * P, num_idxs_reg=gn * P, elem_size=F,
                            single_packet=False, queue_num=gi % NQ)
                        xg_t[gi] = xg
                    xgr = xg_t[gi][:, go:go + n_t, :, :]
                    xgr_t[k] = xgr
                    # 2. xr gather via one-hot matmuls (new pout on first)
                    pxr = ps_xr_p.tile([P, gch, F], f32, tag="pxr")
                    for g in range(n_t):
                        nc.tensor.matmul(pxr[:, g, :], lhsT=ohT_sb[:, g0 + g, :],
                                         rhs=xr_sb[:, b, :], start=True,
                                         stop=True, skip_group_check=True)
                # 3. scatter matmuls for chunk k-1 (one chunk behind)
                if k > 0:
                    pb, pt0, pn_t, pg0, pfirst, plast = chunks[k - 1][:6]
                    if pfirst:
                        pout = ps_out_p.tile([P, FO], f32, tag="pout")
                        if not fuse_den:
                            pden = ps_den_p.tile([P, H], f32, tag="pden")
                    pmsg = msg_t[k - 1]
                    pex = ex_t[k - 1]
                    for g in range(pn_t):
                        st = pfirst and g == 0
                        sp = plast and g == pn_t - 1
                        nc.tensor.matmul(
                            pout[:], lhsT=oh_sb[:, pg0 + g, :],
                            rhs=pmsg[:, g, :, :].rearrange("p h c -> p (h c)")
                            if not fuse_den else pmsg[:, g, 0, :],
                            start=st, stop=sp, skip_group_check=True)
                        if not fuse_den:
                            nc.tensor.matmul(
                                pden[:], lhsT=oh_sb[:, pg0 + g, :],
                                rhs=pex[:, g, :, 0], start=st, stop=sp,
                                skip_group_check=True)
                    msg_t[k - 1] = ex_t[k - 1] = None
                    # 4. epilogue after the block's last accumulation
                    if plast:
                        epilogue(pb)
                if k < nch:
                    # 5-8. chunk-batched vector chain
                    v = ew.tile([P, gch, H, C], bf16, tag="v")
                    nc.vector.tensor_tensor(
                        out=v[:, :n_t], in0=xgr,
                        in1=pxr[:, :n_t, :].rearrange("p g (h c) -> p g h c",
                                                      c=C),
                        op=ALU.add)
                    e = ew.tile([P, gch, H, C], bf16, tag="e")
                    nc.vector.scalar_tensor_tensor(
                        out=e[:, :n_t], in0=v[:, :n_t], scalar=NEG_SLOPE,
                        in1=v[:, :n_t], op0=ALU.mult, op1=ALU.max)
                    t2 = ew.tile([P, gch, H, C], bf16, tag="t2")
                    nc.vector.tensor_tensor(out=t2[:, :n_t], in0=e[:, :n_t],
                                            in1=attb_sb[:, :n_t], op=ALU.mult)
                    sc = ew.tile([P, gch, H, 1], f32, tag="sc")
                    nc.vector.tensor_reduce(out=sc[:, :n_t, :, 0],
                                            in_=t2[:, :n_t],
                                            axis=AX.X, op=ALU.add)
                    # 9. exp on the scalar engine, broadcast to full width so
                    # the msg multiply stays unit-stride (2x DVE mode)
                    exb = ew.tile([P, gch, H, C], bf16, tag="exb")
                    nc.scalar.activation(
                        exb[:, :n_t],
                        sc[:, :n_t].to_broadcast([P, n_t, H, C]), AF.Exp)
                    ex_t[k] = exb
                    # 10. msg = xg * ex (+ ones column for fused denominator)
                    if fuse_den:
                        msg = ew.tile([P, gch, 1, C + 1], bf16, tag="msg")
                        nc.vector.tensor_tensor(
                            out=msg[:, :n_t, :, :C], in0=xgr,
                            in1=exb[:, :n_t], op=ALU.mult)
                        nc.scalar.copy(msg[:, :n_t, :, C],
                                       exb[:, :n_t, :, 0])
                        msg_t[k] = msg
                    else:
                        msg = ew.tile([P, gch, H, C], bf16, tag="msg")
                        nc.vector.tensor_tensor(
                            out=msg[:, :n_t], in0=xgr,
                            in1=exb[:, :n_t], op=ALU.mult)
                        msg_t[k] = msg

    nc.compile()
    return nc


# ---------------------------------------------------------------------------
# Host orchestration
# ---------------------------------------------------------------------------

def _rep(v, gch=1):
    """Replicate a 1-D param vector across 128 partitions (x gch copies)."""
    v = np.asarray(v, dtype=np.float32).reshape(-1)
    if gch > 1:
        v = np.tile(v, gch)
    return np.tile(v[None, :], (P, 1)).astype(BF16)


TRACE = False          # set by test harness to capture NTFF profiles
LAST_RESULTS = []      # BassKernelResults of the last kernel() call

GCH1 = 3               # edge-chunk tiles, layer 1 (PSUM: 2*3*2KB pxr)
GCH2 = 8               # layer 2 (F=128: 2*8*0.5KB pxr)
NQ = 4                 # SWDGE queues used by gathers (round-robin)


def run_spmd(nc, in_maps, trace=False, trace_kwargs=None):
    from concourse import bass_utils
    res = bass_utils.run_bass_kernel_spmd(
        nc, in_maps, core_ids=list(range(len(in_maps))), trace=trace or TRACE,
        **(trace_kwargs or {}))
    LAST_RESULTS.append(res)
    return res


def kernel(x, edge_index, W1l, b1l, W1r, b1r, att1, bias1,
           W2l, b2l, W2r, b2r, att2, bias2, Wa, ba, Wrc, brc):
    x = np.asarray(x, dtype=np.float32)
    Tb, per_core = prep_edges(edge_index)
    ones = np.ones((1, P), dtype=BF16)

    nc1 = _get_program(1, tuple(Tb))
    xT = np.ascontiguousarray(x.T).astype(BF16)
    common1 = {
        "xT": xT,
        "wl": np.asarray(W1l, np.float32).astype(BF16),
        "wr": np.asarray(W1r, np.float32).astype(BF16),
        "bl": np.asarray(b1l, np.float32).reshape(1, -1).astype(BF16),
        "br": np.asarray(b1r, np.float32).reshape(1, -1).astype(BF16),
        "attb": _rep(np.asarray(att1, np.float32).reshape(-1), GCH1),
        "biasb": _rep(bias1),
        "ones": ones,
    }
    in_maps = []
    for c in range(NCORES):
        m = dict(common1)
        m["xoT"] = np.ascontiguousarray(
            x[c * NPC:(c + 1) * NPC].T).astype(BF16)
        m["idx"] = per_core[c]["idx"]
        m["oh"] = per_core[c]["oh"]
        m["ohT"] = per_core[c]["ohT"]
        in_maps.append(m)
    LAST_RESULTS.clear()
    res1 = run_spmd(nc1, in_maps)
    h = np.concatenate([np.asarray(res1.results[c]["h_own"])
                        for c in range(NCORES)], axis=0)

    nc2 = _get_program(2, tuple(Tb))
    hT = np.ascontiguousarray(h.T)  # already bf16
    common2 = {
        "xT": hT,
        "wl": np.asarray(W2l, np.float32).astype(BF16),
        "wr": np.asarray(W2r, np.float32).astype(BF16),
        "bl": np.asarray(b2l, np.float32).reshape(1, -1).astype(BF16),
        "br": np.asarray(b2r, np.float32).reshape(1, -1).astype(BF16),
        "attb": _rep(np.asarray(att2, np.float32).reshape(-1), GCH2),
        "biasb": _rep(bias2),
        "ones": ones,
        "headw": np.concatenate(
            [_rep(np.asarray(Wa, np.float32).reshape(-1)),
             _rep(np.asarray(Wrc, np.float32).reshape(-1))], axis=1),
        "headb": np.concatenate(
            [np.full((P, 1), np.float32(np.asarray(ba).reshape(())), np.float32),
             np.full((P, 1), np.float32(np.asarray(brc).reshape(())), np.float32)],
            axis=1),
    }
    in_maps2 = []
    for c in range(NCORES):
        m = dict(common2)
        m["xoT"] = np.ascontiguousarray(h[c * NPC:(c + 1) * NPC].T)
        m["idx"] = per_core[c]["idx"]
        m["oh"] = per_core[c]["oh"]
        m["ohT"] = per_core[c]["ohT"]
        in_maps2.append(m)
    res2 = run_spmd(nc2, in_maps2)
    an = np.concatenate([np.asarray(res2.results[c]["an"])
                         for c in range(NCORES)], axis=0)
    rc = np.concatenate([np.asarray(res2.results[c]["rc"])
                         for c in range(NCORES)], axis=0)
    # device returns pre-sigmoid logits; finish the heads here
    an = 1.0 / (1.0 + np.exp(-(an + np.float32(np.asarray(ba).reshape(())))))
    rc = 1.0 / (1.0 + np.exp(-(rc + np.float32(np.asarray(brc).reshape(())))))
    return an, rc


_PROGRAMS = {}


def _get_program(layer, tb_key):
    key = (layer, tb_key)
    if key not in _PROGRAMS:
        if layer == 1:
            _PROGRAMS[key] = build_layer(N_NODES, NPC, IN_DIM, HEADS,
                                         list(tb_key), elu=True,
                                         heads_out=False, gch=GCH1)
        else:
            _PROGRAMS[key] = build_layer(N_NODES, NPC, HEADS * HID, 1,
                                         list(tb_key), elu=False,
                                         heads_out=True, gch=GCH2)
    return _PROGRAMS[key]


# revision 38
# speedup vs baseline: 1.0143x; 1.0143x over previous
"""Trainium2 Bass kernel for a 2-layer GATv2 (DependencyGraphAnalyzer).

Strategy (8 cores, SPMD, edge-parallel by dst range):
  - Host sorts edges by dst and shards them by dst-node range: core c owns
    nodes [c*2500, (c+1)*2500) and every edge pointing into that range, so
    softmax segments are core-local.
  - All compute in bf16 (fp32 PSUM accumulation); one-hot scatter/gather
    matrices are precomputed on the host in fp8 and kept SBUF-resident.
  - Dense phase: every core computes xl = x@Wl+bl for ALL nodes (replicated,
    cheap in bf16) and writes it to a DRAM table in bf16; xr for its own
    2500 nodes stays in SBUF.
  - Edge phase per 128-dst-node block, in chunks of GCH 128-edge tiles:
    dma_gather of xl[src] rows (bf16, 4 SWDGE queues round-robin), one-hot
    matmul for xr[dst], then chunk-batched vector ops:
      v = xg + pxr; e = max(.2v, v); t2 = e*att; sc = reduce_c(t2);
      ex = exp(sc); msg = xg * ex
    and per-tile scatter matmuls pout += oh@msg, pden += oh@ex.
    Scatter matmuls run one chunk behind (software pipeline) so the
    in-order tensor queue never waits on the vector chain.
  - Softmax max-subtraction is skipped (scores are O(1); fp32 exp is safe).
  - Two launches; host all-gathers the bf16 hidden state h between layers.
"""

import numpy as np
import ml_dtypes

# Problem constants (hardcoded; kernel.py must be self-contained).
N_NODES = 20000
N_EDGES = 320000
IN_DIM = 256
HID = 128
HEADS = 4
NEG_SLOPE = 0.2
NCORES = 8
NPC = N_NODES // NCORES  # 2500 own nodes per core
P = 128

BF16 = ml_dtypes.bfloat16
FP8 = ml_dtypes.float8_e4m3


# ---------------------------------------------------------------------------
# Host-side edge preprocessing
# ---------------------------------------------------------------------------

def prep_edges(edge_index, n_nodes=N_NODES, ncores=NCORES):
    """Sort edges by dst, shard by dst range, pad each (core, block) segment
    to a common per-block tile count, and build the device index arrays.

    Returns (Tb, per_core): Tb[b] = number of 128-edge tiles of block b
    (shared by all cores); per_core[c] holds:
      idx : [128, TT*8] int16  wrapped dma_gather indices (pad -> n_nodes)
      oh  : [128, TT*128] fp8  oh[p, t*128+j]  = (dst_local[t*128+p] == j)
      ohT : [128, TT*128] fp8  ohT[j, t*128+p] = (dst_local[t*128+p] == j)
    """
    npc = n_nodes // ncores
    nb = (npc + P - 1) // P
    src = np.asarray(edge_index[0], dtype=np.int64)
    dst = np.asarray(edge_index[1], dtype=np.int64)
    order = np.argsort(dst, kind="stable")
    src_s, dst_s = src[order], dst[order]

    core_of = dst_s // npc
    blk_of = core_of * nb + (dst_s - core_of * npc) // P
    counts = np.bincount(blk_of, minlength=ncores * nb).reshape(ncores, nb)
    ends = np.cumsum(counts.reshape(-1)).reshape(ncores, nb)
    starts = ends - counts

    tiles = (counts + P - 1) // P
    Tb = np.maximum(tiles.max(axis=0), 1)
    TT = int(Tb.sum())
    offs = np.concatenate([[0], np.cumsum(Tb)[:-1]])

    jj = np.arange(P, dtype=np.int32)
    per_core = []
    for c in range(ncores):
        idx_flat = np.full(TT * P, n_nodes, dtype=np.int64)  # pad -> zero row
        dloc_flat = np.full(TT * P, -1, dtype=np.int32)      # pad -> -1
        for b in range(nb):
            s, e = starts[c, b], ends[c, b]
            cnt = e - s
            o = offs[b] * P
            idx_flat[o:o + cnt] = src_s[s:e]
            dloc_flat[o:o + cnt] = (dst_s[s:e] - c * npc - b * P).astype(np.int32)
        # Wrap gather indices per block: within a gather call of n idxs,
        # index j lives at [j % 16, j // 16]; replicate rows to 128 parts.
        idx_w = np.zeros((P, TT * 8), dtype=np.int16)
        for b in range(nb):
            o = offs[b] * P
            n = int(Tb[b]) * P
            seg = idx_flat[o:o + n].astype(np.int16)
            w = seg.reshape(n // 16, 16).T  # [16, n/16]
            idx_w[:, o // 16:(o + n) // 16] = np.tile(w, (8, 1))
        # One-hot tables, fp8. E[t, p, j] = (dloc[t*128+p] == j)
        dl = dloc_flat.reshape(TT, P)
        E = (dl[:, :, None] == jj[None, None, :])
        oh = np.ascontiguousarray(
            E.transpose(1, 0, 2).reshape(P, TT * P)).astype(FP8)
        ohT = np.ascontiguousarray(
            E.transpose(2, 0, 1).reshape(P, TT * P)).astype(FP8)
        per_core.append({"idx": idx_w, "oh": oh, "ohT": ohT})
    return [int(t) for t in Tb], per_core


# ---------------------------------------------------------------------------
# Bass program builder (one GATv2 layer, optionally + heads)
# ---------------------------------------------------------------------------

def build_layer(nn, npc, in_dim, heads, Tb, elu, heads_out, gch,
                num_devices=NCORES):
    """Build + compile one layer program. Returns the Bacc object."""
    import concourse.bacc as bacc
    import concourse.tile as tile
    import concourse.mybir as mybir
    from contextlib import ExitStack

    f32 = mybir.dt.float32
    bf16 = mybir.dt.bfloat16
    fp8 = mybir.dt.float8e4
    i16 = mybir.dt.int16
    AF = mybir.ActivationFunctionType
    ALU = mybir.AluOpType
    AX = mybir.AxisListType

    C = HID
    H = heads
    F = H * C
    KC = in_dim // P
    NB = len(Tb)
    TT = sum(Tb)
    blk_rows = [min(P, npc - b * P) for b in range(NB)]
    n_node_tiles = (nn + P - 1) // P
    # L2 folds the denominator into pout as an extra ones-column (F+1 cols).
    import os
    fuse_den = (H == 1) and os.environ.get("NOFUSE", "") != "1"
    FO = F + 1 if fuse_den else F

    nc = bacc.Bacc("TRN2", target_bir_lowering=False, debug=False,
                   num_devices=num_devices, num_swdge_queues=4)

    # --- DRAM tensors ---
    xT = nc.dram_tensor("xT", [in_dim, nn], bf16, kind="ExternalInput").ap()
    xoT = nc.dram_tensor("xoT", [in_dim, npc], bf16, kind="ExternalInput").ap()
    wl = nc.dram_tensor("wl", [in_dim, F], bf16, kind="ExternalInput").ap()
    wr = nc.dram_tensor("wr", [in_dim, F], bf16, kind="ExternalInput").ap()
    bl = nc.dram_tensor("bl", [1, F], bf16, kind="ExternalInput").ap()
    br = nc.dram_tensor("br", [1, F], bf16, kind="ExternalInput").ap()
    attb = nc.dram_tensor("attb", [P, gch * F], bf16, kind="ExternalInput").ap()
    biasb = nc.dram_tensor("biasb", [P, F], bf16, kind="ExternalInput").ap()
    ones_d = nc.dram_tensor("ones", [1, P], bf16, kind="ExternalInput").ap()
    idx_d = nc.dram_tensor("idx", [P, TT * 8], i16, kind="ExternalInput").ap()
    oh_d = nc.dram_tensor("oh", [P, TT * P], fp8, kind="ExternalInput").ap()
    ohT_d = nc.dram_tensor("ohT", [P, TT * P], fp8, kind="ExternalInput").ap()
    if heads_out:
        headw = nc.dram_tensor("headw", [P, 2 * C], bf16,
                               kind="ExternalInput").ap()
        headb = nc.dram_tensor("headb", [P, 2], f32, kind="ExternalInput").ap()
        an_d = nc.dram_tensor("an", [npc, 1], f32, kind="ExternalOutput").ap()
        rc_d = nc.dram_tensor("rc", [npc, 1], f32, kind="ExternalOutput").ap()
    else:
        h_d = nc.dram_tensor("h_own", [npc, F], bf16, kind="ExternalOutput").ap()

    xl_full = nc.dram_tensor("xl_full", [nn + 1, F], bf16, kind="Internal").ap()

    with tile.TileContext(nc) as tc, ExitStack() as es:
        cp = es.enter_context(tc.tile_pool(name="const", bufs=1))

        wl_sb = cp.tile([P, KC, F], bf16, tag="wl")
        nc.sync.dma_start(wl_sb[:], wl.rearrange("(c k) f -> k c f", k=P))
        wr_sb = cp.tile([P, KC, F], bf16, tag="wr")
        nc.sync.dma_start(wr_sb[:], wr.rearrange("(c k) f -> k c f", k=P))
        bl_sb = cp.tile([1, F], bf16, tag="bl")
        nc.sync.dma_start(bl_sb[:], bl[:])
        br_sb = cp.tile([1, F], bf16, tag="br")
        nc.sync.dma_start(br_sb[:], br[:])
        attb_sb = cp.tile([P, gch, H, C], bf16, tag="attb")
        nc.sync.dma_start(attb_sb[:],
                          attb.rearrange("p (g h c) -> p g h c", g=gch, h=H))
        biasb_sb = cp.tile([P, H, C], bf16, tag="biasb")
        nc.sync.dma_start(biasb_sb[:], biasb.rearrange("p (h c) -> p h c", h=H))
        ones_sb = cp.tile([1, P], bf16, tag="ones")
        nc.sync.dma_start(ones_sb[:], ones_d[:])
        idx_sb = cp.tile([P, TT * 8], i16, tag="idx")
        nc.sync.dma_start(idx_sb[:], idx_d[:])
        oh_sb = cp.tile([P, TT, P], fp8, tag="oh")
        nc.sync.dma_start(oh_sb[:], oh_d.rearrange("p (t j) -> p t j", j=P))
        ohT_sb = cp.tile([P, TT, P], fp8, tag="ohT")
        nc.sync.dma_start(ohT_sb[:], ohT_d.rearrange("p (t j) -> p t j", j=P))
        if heads_out:
            headw_sb = cp.tile([P, 2 * C], bf16, tag="headw")
            nc.sync.dma_start(headw_sb[:], headw[:])
            headb_sb = cp.tile([P, 2], f32, tag="headb")
            nc.sync.dma_start(headb_sb[:], headb[:])

        xr_sb = cp.tile([P, NB, F], bf16, tag="xr")
        nc.vector.memset(xr_sb[:], 0.0)

        # ---- dense phase: xl_full = x @ wl + bl (all nodes), xr (own) ----
        # MT node tiles share one DMA load/store (HWDGE sequencer is the
        # dense-phase bottleneck at ~0.7us per dma_start).
        MT = 4
        with tc.tile_pool(name="dpsum", bufs=2, space="PSUM") as dps, \
             tc.tile_pool(name="dwork", bufs=3) as dw:
            for m0 in range(0, n_node_tiles, MT):
                mte = min(m0 + MT, n_node_tiles)
                rr = min(MT * P, nn - m0 * P)
                lt = dw.tile([P, KC, MT * P], bf16, tag="lt")
                nc.sync.dma_start(
                    lt[:, :, :rr],
                    xT[:, m0 * P:m0 * P + rr].rearrange("(c k) n -> k c n", k=P))
                ot = dw.tile([P, MT, F], bf16, tag="ot")
                for m in range(m0, mte):
                    r = min(P, nn - m * P)
                    t = m - m0
                    ps = dps.tile([P, F], f32, tag="ps")
                    for c in range(KC):
                        nc.tensor.matmul(
                            ps[:r], lhsT=lt[:, c, t * P:t * P + r],
                            rhs=wl_sb[:, c, :], start=(c == 0), stop=False)
                    nc.tensor.matmul(ps[:r], lhsT=ones_sb[:, :r], rhs=bl_sb[:],
                                     start=False, stop=True)
                    nc.scalar.copy(ot[:r, t, :], ps[:r])
                if rr == (mte - m0) * P:
                    nc.sync.dma_start(
                        xl_full[m0 * P:m0 * P + rr, :].rearrange(
                            "(t p) f -> p t f", p=P),
                        ot[:, :mte - m0, :])
                else:
                    for m in range(m0, mte):
                        r = min(P, nn - m * P)
                        nc.sync.dma_start(xl_full[m * P:m * P + r, :],
                                          ot[:r, m - m0, :])
            zt = dw.tile([1, F], bf16, tag="zt")
            nc.vector.memset(zt[:], 0.0)
            nc.sync.dma_start(xl_full[nn:nn + 1, :], zt[:])

            for b0 in range(0, NB, MT):
                bte = min(b0 + MT, NB)
                rr = min(MT * P, npc - b0 * P)
                lt = dw.tile([P, KC, MT * P], bf16, tag="lt")
                nc.sync.dma_start(
                    lt[:, :, :rr],
                    xoT[:, b0 * P:b0 * P + rr].rearrange("(c k) n -> k c n",
                                                         k=P))
                for b in range(b0, bte):
                    r = blk_rows[b]
                    t = b - b0
                    ps = dps.tile([P, F], f32, tag="ps")
                    for c in range(KC):
                        nc.tensor.matmul(
                            ps[:r], lhsT=lt[:, c, t * P:t * P + r],
                            rhs=wr_sb[:, c, :], start=(c == 0), stop=False)
                    nc.tensor.matmul(ps[:r], lhsT=ones_sb[:, :r], rhs=br_sb[:],
                                     start=False, stop=True)
                    nc.vector.tensor_copy(xr_sb[:r, b, :], ps[:r])

        tc.strict_bb_all_engine_barrier()

        # ---- edge phase ----
        # compute chunks: (block, tile0, ntiles, global tile offset, first,
        #                  last, gather chunk idx, offset within gather chunk)
        # gather chunks: (global tile offset, ntiles) of up to GG tiles
        GG = 9
        chunks = []
        gchunks = []
        off = 0
        for b in range(NB):
            for g0 in range(0, Tb[b], GG):
                gn = min(GG, Tb[b] - g0)
                gi = len(gchunks)
                gchunks.append((off + g0, gn))
                for t0 in range(g0, g0 + gn, gch):
                    n_t = min(gch, g0 + gn - t0)
                    chunks.append((b, t0, n_t, off + t0,
                                   t0 == 0, t0 + n_t == Tb[b], gi, t0 - g0))
            off += Tb[b]
        nch = len(chunks)

        with tc.tile_pool(name="gxg", bufs=3) as gxg, \
             tc.tile_pool(name="ew", bufs=3) as ew, \
             tc.tile_pool(name="ep", bufs=2) as epp, \
             tc.tile_pool(name="ps_xr", bufs=2, space="PSUM") as ps_xr_p, \
             tc.tile_pool(name="ps_out", bufs=1, space="PSUM") as ps_out_p, \
             tc.tile_pool(name="ps_den", bufs=1, space="PSUM") as ps_den_p:

            xg_t = [None] * len(gchunks)
            msg_t = [None] * nch
            ex_t = [None] * nch
            xgr_t = [None] * nch
            pout = pden = None

            def epilogue(b):
                r = blk_rows[b]
                if fuse_den:
                    den = epp.tile([P, 1], f32, tag="den")
                    nc.vector.tensor_scalar_add(den[:], pout[:, F:F + 1], 1e-16)
                    rec = epp.tile([P, 1, 1], f32, tag="rec")
                    nc.vector.reciprocal(rec[:, 0, :], den[:])
                    recb = rec[:].to_broadcast([P, H, C])
                else:
                    den = epp.tile([P, H, 1], f32, tag="den")
                    nc.vector.tensor_scalar_add(den[:, :, 0], pden[:], 1e-16)
                    rec = epp.tile([P, H, 1], f32, tag="rec")
                    nc.vector.reciprocal(rec[:], den[:])
                    recb = rec[:].to_broadcast([P, H, C])
                hb = epp.tile([P, H, C], bf16, tag="hb")
                nc.vector.tensor_tensor(
                    out=hb[:], in0=pout[:, :F].rearrange("p (h c) -> p h c", c=C),
                    in1=recb, op=ALU.mult)
                hc = epp.tile([P, H, C], bf16, tag="hc")
                nc.vector.tensor_tensor(out=hc[:], in0=hb[:], in1=biasb_sb[:],
                                        op=ALU.add)
                if elu:
                    mn = epp.tile([P, H, C], bf16, tag="mn")
                    nc.vector.tensor_scalar_min(mn[:], hc[:], 0.0)
                    en = epp.tile([P, H, C], f32, tag="en")
                    nc.scalar.activation(en[:], mn[:], AF.Exp)
                    mx = epp.tile([P, H, C], bf16, tag="mx")
                    nc.vector.tensor_scalar_max(mx[:], hc[:], 0.0)
                    ho = epp.tile([P, H, C], bf16, tag="ho")
                    nc.vector.scalar_tensor_tensor(
                        out=ho[:], in0=mx[:], scalar=-1.0, in1=en[:],
                        op0=ALU.add, op1=ALU.add)
                    nc.sync.dma_start(
                        h_d[b * P:b * P + r, :],
                        ho[:r].rearrange("p h c -> p (h c)"))
                elif heads_out:
                    # logits only; sigmoid is applied on the host
                    for j, outd in enumerate([an_d, rc_d]):
                        scr = epp.tile([P, C], bf16, tag="scr")
                        nc.vector.tensor_tensor(
                            out=scr[:], in0=hc[:, 0, :],
                            in1=headw_sb[:, j * C:(j + 1) * C], op=ALU.mult)
                        red = epp.tile([P, 1], f32, tag="red")
                        nc.vector.tensor_reduce(out=red[:], in_=scr[:],
                                                axis=AX.X, op=ALU.add)
                        nc.sync.dma_start(outd[b * P:b * P + r, :], red[:r])
                else:
                    nc.sync.dma_start(
                        h_d[b * P:b * P + r, :],
                        hc[:r].rearrange("p h c -> p (h c)"))

            for k in range(nch + 1):
                if k < nch:
                    b, t0, n_t, g0, first, last, gi, go = chunks[k]
                    # 1. gather xl[src] rows, GG tiles per call (4 queues, rr)
                    if xg_t[gi] is None:
                        gg0, gn = gchunks[gi]
                        xg = gxg.tile([P, GG, H, C], bf16, tag="xg")
                        nc.gpsimd.dma_gather(
                            xg[:, :gn, :, :].rearrange("p g h c -> p g (h c)"),
                            xl_full[:], idx_sb[:, gg0 * 8:(gg0 + gn) * 8],
                            num_idxs=gn * P, num_idxs_reg=gn * P, elem_size=F,
                            single_packet=False, queue_num=gi % NQ)
                        xg_t[gi] = xg
                    xgr = xg_t[gi][:, go:go + n_t, :, :]
                    xgr_t[k] = xgr
                    # 2. xr gather via one-hot matmuls (new pout on first)
                    pxr = ps_xr_p.tile([P, gch, F], f32, tag="pxr")
                    for g in range(n_t):
                        nc.tensor.matmul(pxr[:, g, :], lhsT=ohT_sb[:, g0 + g, :],
                                         rhs=xr_sb[:, b, :], start=True,
                                         stop=True, skip_group_check=True)
                # 3. scatter matmuls for chunk k-1 (one chunk behind)
                if k > 0:
                    pb, pt0, pn_t, pg0, pfirst, plast = chunks[k - 1][:6]
                    if pfirst:
                        pout = ps_out_p.tile([P, FO], f32, tag="pout")
                        if not fuse_den:
                            pden = ps_den_p.tile([P, H], f32, tag="pden")
                    pmsg = msg_t[k - 1]
                    pex = ex_t[k - 1]
                    for g in range(pn_t):
                        st = pfirst and g == 0
                        sp = plast and g == pn_t - 1
                        nc.tensor.matmul(
                            pout[:], lhsT=oh_sb[:, pg0 + g, :],
                            rhs=pmsg[:, g, :, :].rearrange("p h c -> p (h c)")
                            if not fuse_den else pmsg[:, g, 0, :],
                            start=st, stop=sp, skip_group_check=True)
                        if not fuse_den:
                            nc.tensor.matmul(
                                pden[:], lhsT=oh_sb[:, pg0 + g, :],
                                rhs=pex[:, g, :, 0], start=st, stop=sp,
                                skip_group_check=True)
                    msg_t[k - 1] = ex_t[k - 1] = None
                    # 4. epilogue after the block's last accumulation
                    if plast:
                        epilogue(pb)
                if k < nch:
                    # 5-8. chunk-batched vector chain
                    v = ew.tile([P, gch, H, C], bf16, tag="v")
                    nc.vector.tensor_tensor(
                        out=v[:, :n_t], in0=xgr,
                        in1=pxr[:, :n_t, :].rearrange("p g (h c) -> p g h c",
                                                      c=C),
                        op=ALU.add)
                    e = ew.tile([P, gch, H, C], bf16, tag="e")
                    nc.vector.scalar_tensor_tensor(
                        out=e[:, :n_t], in0=v[:, :n_t], scalar=NEG_SLOPE,
                        in1=v[:, :n_t], op0=ALU.mult, op1=ALU.max)
                    t2 = ew.tile([P, gch, H, C], bf16, tag="t2")
                    nc.vector.tensor_tensor(out=t2[:, :n_t], in0=e[:, :n_t],
                                            in1=attb_sb[:, :n_t], op=ALU.mult)
                    sc = ew.tile([P, gch, H, 1], f32, tag="sc")
                    nc.vector.tensor_reduce(out=sc[:, :n_t, :, 0],
                                            in_=t2[:, :n_t],
                                            axis=AX.X, op=ALU.add)
                    # 9. exp on the scalar engine, broadcast to full width so
                    # the msg multiply stays unit-stride (2x DVE mode)
                    if fuse_den:
                        # exp fills C+1 cols of the msg tile; the multiply then
                        # overwrites cols 0..C-1 in place, leaving col C = ex
                        # as the fused-denominator ones-column (no copy).
                        msg = ew.tile([P, gch, 1, C + 1], bf16, tag="msg")
                        nc.scalar.activation(
                            msg[:, :n_t],
                            sc[:, :n_t].to_broadcast([P, n_t, H, C + 1]),
                            AF.Exp)
                        ex_t[k] = msg
                        nc.vector.tensor_tensor(
                            out=msg[:, :n_t, :, :C], in0=xgr,
                            in1=msg[:, :n_t, :, :C], op=ALU.mult)
                        msg_t[k] = msg
                    else:
                        exb = ew.tile([P, gch, H, C], bf16, tag="exb")
                        nc.scalar.activation(
                            exb[:, :n_t],
                            sc[:, :n_t].to_broadcast([P, n_t, H, C]), AF.Exp)
                        ex_t[k] = exb
                        msg = ew.tile([P, gch, H, C], bf16, tag="msg")
                        nc.vector.tensor_tensor(
                            out=msg[:, :n_t], in0=xgr,
                            in1=exb[:, :n_t], op=ALU.mult)
                        msg_t[k] = msg
                        del exb

    nc.compile()
    return nc


# ---------------------------------------------------------------------------
# Host orchestration
# ---------------------------------------------------------------------------

def _rep(v, gch=1):
    """Replicate a 1-D param vector across 128 partitions (x gch copies)."""
    v = np.asarray(v, dtype=np.float32).reshape(-1)
    if gch > 1:
        v = np.tile(v, gch)
    return np.tile(v[None, :], (P, 1)).astype(BF16)


TRACE = False          # set by test harness to capture NTFF profiles
LAST_RESULTS = []      # BassKernelResults of the last kernel() call

GCH1 = 3               # edge-chunk tiles, layer 1 (PSUM: 2*3*2KB pxr)
GCH2 = 8               # layer 2 (F=128: 2*8*0.5KB pxr)
NQ = 4                 # SWDGE queues used by gathers (round-robin)


def run_spmd(nc, in_maps, trace=False, trace_kwargs=None):
    from concourse import bass_utils
    res = bass_utils.run_bass_kernel_spmd(
        nc, in_maps, core_ids=list(range(len(in_maps))), trace=trace or TRACE,
        **(trace_kwargs or {}))
    LAST_RESULTS.append(res)
    return res


def kernel(x, edge_index, W1l, b1l, W1r, b1r, att1, bias1,
           W2l, b2l, W2r, b2r, att2, bias2, Wa, ba, Wrc, brc):
    x = np.asarray(x, dtype=np.float32)
    Tb, per_core = prep_edges(edge_index)
    ones = np.ones((1, P), dtype=BF16)

    nc1 = _get_program(1, tuple(Tb))
    xT = np.ascontiguousarray(x.T).astype(BF16)
    common1 = {
        "xT": xT,
        "wl": np.asarray(W1l, np.float32).astype(BF16),
        "wr": np.asarray(W1r, np.float32).astype(BF16),
        "bl": np.asarray(b1l, np.float32).reshape(1, -1).astype(BF16),
        "br": np.asarray(b1r, np.float32).reshape(1, -1).astype(BF16),
        "attb": _rep(np.asarray(att1, np.float32).reshape(-1), GCH1),
        "biasb": _rep(bias1),
        "ones": ones,
    }
    in_maps = []
    for c in range(NCORES):
        m = dict(common1)
        m["xoT"] = np.ascontiguousarray(
            x[c * NPC:(c + 1) * NPC].T).astype(BF16)
        m["idx"] = per_core[c]["idx"]
        m["oh"] = per_core[c]["oh"]
        m["ohT"] = per_core[c]["ohT"]
        in_maps.append(m)
    LAST_RESULTS.clear()
    res1 = run_spmd(nc1, in_maps)
    h = np.concatenate([np.asarray(res1.results[c]["h_own"])
                        for c in range(NCORES)], axis=0)

    nc2 = _get_program(2, tuple(Tb))
    hT = np.ascontiguousarray(h.T)  # already bf16
    common2 = {
        "xT": hT,
        "wl": np.asarray(W2l, np.float32).astype(BF16),
        "wr": np.asarray(W2r, np.float32).astype(BF16),
        "bl": np.asarray(b2l, np.float32).reshape(1, -1).astype(BF16),
        "br": np.asarray(b2r, np.float32).reshape(1, -1).astype(BF16),
        "attb": _rep(np.asarray(att2, np.float32).reshape(-1), GCH2),
        "biasb": _rep(bias2),
        "ones": ones,
        "headw": np.concatenate(
            [_rep(np.asarray(Wa, np.float32).reshape(-1)),
             _rep(np.asarray(Wrc, np.float32).reshape(-1))], axis=1),
        "headb": np.concatenate(
            [np.full((P, 1), np.float32(np.asarray(ba).reshape(())), np.float32),
             np.full((P, 1), np.float32(np.asarray(brc).reshape(())), np.float32)],
            axis=1),
    }
    in_maps2 = []
    for c in range(NCORES):
        m = dict(common2)
        m["xoT"] = np.ascontiguousarray(h[c * NPC:(c + 1) * NPC].T)
        m["idx"] = per_core[c]["idx"]
        m["oh"] = per_core[c]["oh"]
        m["ohT"] = per_core[c]["ohT"]
        in_maps2.append(m)
    res2 = run_spmd(nc2, in_maps2)
    an = np.concatenate([np.asarray(res2.results[c]["an"])
                         for c in range(NCORES)], axis=0)
    rc = np.concatenate([np.asarray(res2.results[c]["rc"])
                         for c in range(NCORES)], axis=0)
    # device returns pre-sigmoid logits; finish the heads here
    an = 1.0 / (1.0 + np.exp(-(an + np.float32(np.asarray(ba).reshape(())))))
    rc = 1.0 / (1.0 + np.exp(-(rc + np.float32(np.asarray(brc).reshape(())))))
    return an, rc


_PROGRAMS = {}


def _get_program(layer, tb_key):
    key = (layer, tb_key)
    if key not in _PROGRAMS:
        if layer == 1:
            _PROGRAMS[key] = build_layer(N_NODES, NPC, IN_DIM, HEADS,
                                         list(tb_key), elu=True,
                                         heads_out=False, gch=GCH1)
        else:
            _PROGRAMS[key] = build_layer(N_NODES, NPC, HEADS * HID, 1,
                                         list(tb_key), elu=False,
                                         heads_out=True, gch=GCH2)
    return _PROGRAMS[key]


# revision 42
# speedup vs baseline: 1.0625x; 1.0475x over previous
"""Trainium2 Bass kernel for a 2-layer GATv2 (DependencyGraphAnalyzer).

Strategy (8 cores, SPMD, edge-parallel by dst range):
  - Host sorts edges by dst and shards them by dst-node range: core c owns
    nodes [c*2500, (c+1)*2500) and every edge pointing into that range, so
    softmax segments are core-local.
  - All compute in bf16 (fp32 PSUM accumulation); one-hot scatter/gather
    matrices are precomputed on the host in fp8 and kept SBUF-resident.
  - Dense phase: every core computes xl = x@Wl+bl for ALL nodes (replicated,
    cheap in bf16) and writes it to a DRAM table in bf16; xr for its own
    2500 nodes stays in SBUF.
  - Edge phase per 128-dst-node block, in chunks of GCH 128-edge tiles:
    dma_gather of xl[src] rows (bf16, 4 SWDGE queues round-robin), one-hot
    matmul for xr[dst], then chunk-batched vector ops:
      v = xg + pxr; e = max(.2v, v); t2 = e*att; sc = reduce_c(t2);
      ex = exp(sc); msg = xg * ex
    and per-tile scatter matmuls pout += oh@msg, pden += oh@ex.
    Scatter matmuls run one chunk behind (software pipeline) so the
    in-order tensor queue never waits on the vector chain.
  - Softmax max-subtraction is skipped (scores are O(1); fp32 exp is safe).
  - Two launches; host all-gathers the bf16 hidden state h between layers.
"""

import numpy as np
import ml_dtypes

# Problem constants (hardcoded; kernel.py must be self-contained).
N_NODES = 20000
N_EDGES = 320000
IN_DIM = 256
HID = 128
HEADS = 4
NEG_SLOPE = 0.2
NCORES = 8
NPC = N_NODES // NCORES  # 2500 own nodes per core
P = 128

BF16 = ml_dtypes.bfloat16
FP8 = ml_dtypes.float8_e4m3


# ---------------------------------------------------------------------------
# Host-side edge preprocessing
# ---------------------------------------------------------------------------

def prep_edges(edge_index, n_nodes=N_NODES, ncores=NCORES):
    """Sort edges by dst, shard by dst range, pad each (core, block) segment
    to a common per-block tile count, and build the device index arrays.

    Returns (Tb, per_core): Tb[b] = number of 128-edge tiles of block b
    (shared by all cores); per_core[c] holds:
      idx : [128, TT*8] int16  wrapped dma_gather indices (pad -> n_nodes)
      oh  : [128, TT*128] fp8  oh[p, t*128+j]  = (dst_local[t*128+p] == j)
      ohT : [128, TT*128] fp8  ohT[j, t*128+p] = (dst_local[t*128+p] == j)
    """
    npc = n_nodes // ncores
    nb = (npc + P - 1) // P
    src = np.asarray(edge_index[0], dtype=np.int64)
    dst = np.asarray(edge_index[1], dtype=np.int64)
    order = np.argsort(dst, kind="stable")
    src_s, dst_s = src[order], dst[order]

    core_of = dst_s // npc
    blk_of = core_of * nb + (dst_s - core_of * npc) // P
    counts = np.bincount(blk_of, minlength=ncores * nb).reshape(ncores, nb)
    ends = np.cumsum(counts.reshape(-1)).reshape(ncores, nb)
    starts = ends - counts

    tiles = (counts + P - 1) // P
    Tb = np.maximum(tiles.max(axis=0), 1)
    TT = int(Tb.sum())
    offs = np.concatenate([[0], np.cumsum(Tb)[:-1]])

    jj = np.arange(P, dtype=np.int32)
    per_core = []
    for c in range(ncores):
        idx_flat = np.full(TT * P, n_nodes, dtype=np.int64)  # pad -> zero row
        dloc_flat = np.full(TT * P, -1, dtype=np.int32)      # pad -> -1
        for b in range(nb):
            s, e = starts[c, b], ends[c, b]
            cnt = e - s
            o = offs[b] * P
            idx_flat[o:o + cnt] = src_s[s:e]
            dloc_flat[o:o + cnt] = (dst_s[s:e] - c * npc - b * P).astype(np.int32)
        # Wrap gather indices per block: within a gather call of n idxs,
        # index j lives at [j % 16, j // 16]; replicate rows to 128 parts.
        idx_w = np.zeros((P, TT * 8), dtype=np.int16)
        for b in range(nb):
            o = offs[b] * P
            n = int(Tb[b]) * P
            seg = idx_flat[o:o + n].astype(np.int16)
            w = seg.reshape(n // 16, 16).T  # [16, n/16]
            idx_w[:, o // 16:(o + n) // 16] = np.tile(w, (8, 1))
        # One-hot tables, fp8. E[t, p, j] = (dloc[t*128+p] == j)
        dl = dloc_flat.reshape(TT, P)
        E = (dl[:, :, None] == jj[None, None, :])
        oh = np.ascontiguousarray(
            E.transpose(1, 0, 2).reshape(P, TT * P)).astype(FP8)
        ohT = np.ascontiguousarray(
            E.transpose(2, 0, 1).reshape(P, TT * P)).astype(FP8)
        per_core.append({"idx": idx_w, "oh": oh, "ohT": ohT})
    return [int(t) for t in Tb], per_core


# ---------------------------------------------------------------------------
# Bass program builder (one GATv2 layer, optionally + heads)
# ---------------------------------------------------------------------------

def build_layer(nn, npc, in_dim, heads, Tb, elu, heads_out, gch,
                num_devices=NCORES):
    """Build + compile one layer program. Returns the Bacc object."""
    import concourse.bacc as bacc
    import concourse.tile as tile
    import concourse.mybir as mybir
    from contextlib import ExitStack

    f32 = mybir.dt.float32
    bf16 = mybir.dt.bfloat16
    fp8 = mybir.dt.float8e4
    i16 = mybir.dt.int16
    AF = mybir.ActivationFunctionType
    ALU = mybir.AluOpType
    AX = mybir.AxisListType

    C = HID
    H = heads
    F = H * C
    KC = in_dim // P
    NB = len(Tb)
    TT = sum(Tb)
    blk_rows = [min(P, npc - b * P) for b in range(NB)]
    n_node_tiles = (nn + P - 1) // P
    # L2 folds the denominator into pout as an extra ones-column (F+1 cols).
    import os
    fuse_den = (H == 1) and os.environ.get("NOFUSE", "") != "1"
    FO = F + 1 if fuse_den else F

    nc = bacc.Bacc("TRN2", target_bir_lowering=False, debug=False,
                   num_devices=num_devices, num_swdge_queues=4)

    # --- DRAM tensors ---
    xT = nc.dram_tensor("xT", [in_dim, nn], bf16, kind="ExternalInput").ap()
    xoT = nc.dram_tensor("xoT", [in_dim, npc], bf16, kind="ExternalInput").ap()
    wl = nc.dram_tensor("wl", [in_dim, F], bf16, kind="ExternalInput").ap()
    wr = nc.dram_tensor("wr", [in_dim, F], bf16, kind="ExternalInput").ap()
    bl = nc.dram_tensor("bl", [1, F], bf16, kind="ExternalInput").ap()
    br = nc.dram_tensor("br", [1, F], bf16, kind="ExternalInput").ap()
    attb = nc.dram_tensor("attb", [P, gch * F], bf16, kind="ExternalInput").ap()
    biasb = nc.dram_tensor("biasb", [P, F], bf16, kind="ExternalInput").ap()
    ones_d = nc.dram_tensor("ones", [1, P], bf16, kind="ExternalInput").ap()
    idx_d = nc.dram_tensor("idx", [P, TT * 8], i16, kind="ExternalInput").ap()
    oh_d = nc.dram_tensor("oh", [P, TT * P], fp8, kind="ExternalInput").ap()
    ohT_d = nc.dram_tensor("ohT", [P, TT * P], fp8, kind="ExternalInput").ap()
    if heads_out:
        headw = nc.dram_tensor("headw", [P, 2 * C], bf16,
                               kind="ExternalInput").ap()
        headb = nc.dram_tensor("headb", [P, 2], f32, kind="ExternalInput").ap()
        an_d = nc.dram_tensor("an", [npc, 1], f32, kind="ExternalOutput").ap()
        rc_d = nc.dram_tensor("rc", [npc, 1], f32, kind="ExternalOutput").ap()
    else:
        h_d = nc.dram_tensor("h_own", [npc, F], bf16, kind="ExternalOutput").ap()

    xl_full = nc.dram_tensor("xl_full", [nn + 1, F], bf16, kind="Internal").ap()

    with tile.TileContext(nc) as tc, ExitStack() as es:
        cp = es.enter_context(tc.tile_pool(name="const", bufs=1))

        wl_sb = cp.tile([P, KC, F], bf16, tag="wl")
        nc.sync.dma_start(wl_sb[:], wl.rearrange("(c k) f -> k c f", k=P))
        wr_sb = cp.tile([P, KC, F], bf16, tag="wr")
        nc.sync.dma_start(wr_sb[:], wr.rearrange("(c k) f -> k c f", k=P))
        bl_sb = cp.tile([1, F], bf16, tag="bl")
        nc.sync.dma_start(bl_sb[:], bl[:])
        br_sb = cp.tile([1, F], bf16, tag="br")
        nc.sync.dma_start(br_sb[:], br[:])
        attb_sb = cp.tile([P, gch, H, C], bf16, tag="attb")
        nc.sync.dma_start(attb_sb[:],
                          attb.rearrange("p (g h c) -> p g h c", g=gch, h=H))
        biasb_sb = cp.tile([P, H, C], bf16, tag="biasb")
        nc.sync.dma_start(biasb_sb[:], biasb.rearrange("p (h c) -> p h c", h=H))
        ones_sb = cp.tile([1, P], bf16, tag="ones")
        nc.sync.dma_start(ones_sb[:], ones_d[:])
        idx_sb = cp.tile([P, TT * 8], i16, tag="idx")
        nc.sync.dma_start(idx_sb[:], idx_d[:])
        oh_sb = cp.tile([P, TT, P], fp8, tag="oh")
        nc.sync.dma_start(oh_sb[:], oh_d.rearrange("p (t j) -> p t j", j=P))
        ohT_sb = cp.tile([P, TT, P], fp8, tag="ohT")
        nc.sync.dma_start(ohT_sb[:], ohT_d.rearrange("p (t j) -> p t j", j=P))
        if heads_out:
            headw_sb = cp.tile([P, 2 * C], bf16, tag="headw")
            nc.sync.dma_start(headw_sb[:], headw[:])
            headb_sb = cp.tile([P, 2], f32, tag="headb")
            nc.sync.dma_start(headb_sb[:], headb[:])

        xr_sb = cp.tile([P, NB, F], bf16, tag="xr")
        nc.vector.memset(xr_sb[:], 0.0)

        # ---- dense phase: xl_full = x @ wl + bl (all nodes), xr (own) ----
        # MT node tiles share one DMA load/store (HWDGE sequencer is the
        # dense-phase bottleneck at ~0.7us per dma_start).
        MT = 4
        with tc.tile_pool(name="dpsum", bufs=2, space="PSUM") as dps, \
             tc.tile_pool(name="dwork", bufs=3) as dw:
            for m0 in range(0, n_node_tiles, MT):
                mte = min(m0 + MT, n_node_tiles)
                rr = min(MT * P, nn - m0 * P)
                lt = dw.tile([P, KC, MT * P], bf16, tag="lt")
                nc.sync.dma_start(
                    lt[:, :, :rr],
                    xT[:, m0 * P:m0 * P + rr].rearrange("(c k) n -> k c n", k=P))
                ot = dw.tile([P, MT, F], bf16, tag="ot")
                for m in range(m0, mte):
                    r = min(P, nn - m * P)
                    t = m - m0
                    ps = dps.tile([P, F], f32, tag="ps")
                    for c in range(KC):
                        nc.tensor.matmul(
                            ps[:r], lhsT=lt[:, c, t * P:t * P + r],
                            rhs=wl_sb[:, c, :], start=(c == 0), stop=False)
                    nc.tensor.matmul(ps[:r], lhsT=ones_sb[:, :r], rhs=bl_sb[:],
                                     start=False, stop=True)
                    nc.scalar.copy(ot[:r, t, :], ps[:r])
                if rr == (mte - m0) * P:
                    nc.sync.dma_start(
                        xl_full[m0 * P:m0 * P + rr, :].rearrange(
                            "(t p) f -> p t f", p=P),
                        ot[:, :mte - m0, :])
                else:
                    for m in range(m0, mte):
                        r = min(P, nn - m * P)
                        nc.sync.dma_start(xl_full[m * P:m * P + r, :],
                                          ot[:r, m - m0, :])
            zt = dw.tile([1, F], bf16, tag="zt")
            nc.vector.memset(zt[:], 0.0)
            nc.sync.dma_start(xl_full[nn:nn + 1, :], zt[:])

            for b0 in range(0, NB, MT):
                bte = min(b0 + MT, NB)
                rr = min(MT * P, npc - b0 * P)
                lt = dw.tile([P, KC, MT * P], bf16, tag="lt")
                nc.sync.dma_start(
                    lt[:, :, :rr],
                    xoT[:, b0 * P:b0 * P + rr].rearrange("(c k) n -> k c n",
                                                         k=P))
                for b in range(b0, bte):
                    r = blk_rows[b]
                    t = b - b0
                    ps = dps.tile([P, F], f32, tag="ps")
                    for c in range(KC):
                        nc.tensor.matmul(
                            ps[:r], lhsT=lt[:, c, t * P:t * P + r],
                            rhs=wr_sb[:, c, :], start=(c == 0), stop=False)
                    nc.tensor.matmul(ps[:r], lhsT=ones_sb[:, :r], rhs=br_sb[:],
                                     start=False, stop=True)
                    nc.vector.tensor_copy(xr_sb[:r, b, :], ps[:r])

        tc.strict_bb_all_engine_barrier()

        # ---- edge phase ----
        # compute chunks: (block, tile0, ntiles, global tile offset, first,
        #                  last, gather chunk idx, offset within gather chunk)
        # gather chunks: (global tile offset, ntiles) of up to GG tiles
        GG = 9
        chunks = []
        gchunks = []
        off = 0
        for b in range(NB):
            for g0 in range(0, Tb[b], GG):
                gn = min(GG, Tb[b] - g0)
                gi = len(gchunks)
                gchunks.append((off + g0, gn))
                for t0 in range(g0, g0 + gn, gch):
                    n_t = min(gch, g0 + gn - t0)
                    chunks.append((b, t0, n_t, off + t0,
                                   t0 == 0, t0 + n_t == Tb[b], gi, t0 - g0))
            off += Tb[b]
        nch = len(chunks)

        with tc.tile_pool(name="gxg", bufs=3) as gxg, \
             tc.tile_pool(name="ew", bufs=2) as ew, \
             tc.tile_pool(name="ep", bufs=1 if F > 128 else 2) as epp, \
             tc.tile_pool(name="ps_xr", bufs=2 if gch * F * 8 <= 12288
                          else 1, space="PSUM") as ps_xr_p, \
             tc.tile_pool(name="ps_out", bufs=1, space="PSUM") as ps_out_p, \
             tc.tile_pool(name="ps_den", bufs=1, space="PSUM") as ps_den_p:

            xg_t = [None] * len(gchunks)
            msg_t = [None] * nch
            ex_t = [None] * nch
            xgr_t = [None] * nch
            pout = pden = None

            def epilogue(b):
                r = blk_rows[b]
                if fuse_den:
                    den = epp.tile([P, 1], f32, tag="den")
                    nc.vector.tensor_scalar_add(den[:], pout[:, F:F + 1], 1e-16)
                    rec = epp.tile([P, 1, 1], f32, tag="rec")
                    nc.vector.reciprocal(rec[:, 0, :], den[:])
                    recb = rec[:].to_broadcast([P, H, C])
                else:
                    den = epp.tile([P, H, 1], f32, tag="den")
                    nc.vector.tensor_scalar_add(den[:, :, 0], pden[:], 1e-16)
                    rec = epp.tile([P, H, 1], f32, tag="rec")
                    nc.vector.reciprocal(rec[:], den[:])
                    recb = rec[:].to_broadcast([P, H, C])
                hb = epp.tile([P, H, C], bf16, tag="hb")
                nc.vector.tensor_tensor(
                    out=hb[:], in0=pout[:, :F].rearrange("p (h c) -> p h c", c=C),
                    in1=recb, op=ALU.mult)
                hc = epp.tile([P, H, C], bf16, tag="hc")
                nc.vector.tensor_tensor(out=hc[:], in0=hb[:], in1=biasb_sb[:],
                                        op=ALU.add)
                if elu:
                    mn = epp.tile([P, H, C], bf16, tag="mn")
                    nc.vector.tensor_scalar_min(mn[:], hc[:], 0.0)
                    en = epp.tile([P, H, C], f32, tag="en")
                    nc.scalar.activation(en[:], mn[:], AF.Exp)
                    mx = epp.tile([P, H, C], bf16, tag="mx")
                    nc.vector.tensor_scalar_max(mx[:], hc[:], 0.0)
                    ho = epp.tile([P, H, C], bf16, tag="ho")
                    nc.vector.scalar_tensor_tensor(
                        out=ho[:], in0=mx[:], scalar=-1.0, in1=en[:],
                        op0=ALU.add, op1=ALU.add)
                    nc.sync.dma_start(
                        h_d[b * P:b * P + r, :],
                        ho[:r].rearrange("p h c -> p (h c)"))
                elif heads_out:
                    # logits only; sigmoid is applied on the host
                    for j, outd in enumerate([an_d, rc_d]):
                        scr = epp.tile([P, C], bf16, tag="scr")
                        nc.vector.tensor_tensor(
                            out=scr[:], in0=hc[:, 0, :],
                            in1=headw_sb[:, j * C:(j + 1) * C], op=ALU.mult)
                        red = epp.tile([P, 1], f32, tag="red")
                        nc.vector.tensor_reduce(out=red[:], in_=scr[:],
                                                axis=AX.X, op=ALU.add)
                        nc.sync.dma_start(outd[b * P:b * P + r, :], red[:r])
                else:
                    nc.sync.dma_start(
                        h_d[b * P:b * P + r, :],
                        hc[:r].rearrange("p h c -> p (h c)"))

            for k in range(nch + 1):
                if k < nch:
                    b, t0, n_t, g0, first, last, gi, go = chunks[k]
                    # 1. gather xl[src] rows, GG tiles per call (4 queues, rr)
                    if xg_t[gi] is None:
                        gg0, gn = gchunks[gi]
                        xg = gxg.tile([P, GG, H, C], bf16, tag="xg")
                        nc.gpsimd.dma_gather(
                            xg[:, :gn, :, :].rearrange("p g h c -> p g (h c)"),
                            xl_full[:], idx_sb[:, gg0 * 8:(gg0 + gn) * 8],
                            num_idxs=gn * P, num_idxs_reg=gn * P, elem_size=F,
                            single_packet=False, queue_num=gi % NQ)
                        xg_t[gi] = xg
                    xgr = xg_t[gi][:, go:go + n_t, :, :]
                    xgr_t[k] = xgr
                    # 2. xr gather via one-hot matmuls (new pout on first)
                    pxr = ps_xr_p.tile([P, gch, F], f32, tag="pxr")
                    for g in range(n_t):
                        nc.tensor.matmul(pxr[:, g, :], lhsT=ohT_sb[:, g0 + g, :],
                                         rhs=xr_sb[:, b, :], start=True,
                                         stop=True, skip_group_check=True)
                # 3. scatter matmuls for chunk k-1 (one chunk behind)
                if k > 0:
                    pb, pt0, pn_t, pg0, pfirst, plast = chunks[k - 1][:6]
                    if pfirst:
                        pout = ps_out_p.tile([P, FO], f32, tag="pout")
                        if not fuse_den:
                            pden = ps_den_p.tile([P, H], f32, tag="pden")
                    pmsg = msg_t[k - 1]
                    pex = ex_t[k - 1]
                    for g in range(pn_t):
                        st = pfirst and g == 0
                        sp = plast and g == pn_t - 1
                        nc.tensor.matmul(
                            pout[:], lhsT=oh_sb[:, pg0 + g, :],
                            rhs=pmsg[:, g, :, :].rearrange("p h c -> p (h c)")
                            if not fuse_den else pmsg[:, g, 0, :],
                            start=st, stop=sp, skip_group_check=True)
                        if not fuse_den:
                            nc.tensor.matmul(
                                pden[:], lhsT=oh_sb[:, pg0 + g, :],
                                rhs=pex[:, g, :, 0], start=st, stop=sp,
                                skip_group_check=True)
                    msg_t[k - 1] = ex_t[k - 1] = None
                    # 4. epilogue after the block's last accumulation
                    if plast:
                        epilogue(pb)
                if k < nch:
                    # 5-8. chunk-batched vector chain
                    v = ew.tile([P, gch, H, C], bf16, tag="v")
                    nc.vector.tensor_tensor(
                        out=v[:, :n_t], in0=xgr,
                        in1=pxr[:, :n_t, :].rearrange("p g (h c) -> p g h c",
                                                      c=C),
                        op=ALU.add)
                    e = ew.tile([P, gch, H, C], bf16, tag="e")
                    nc.vector.scalar_tensor_tensor(
                        out=e[:, :n_t], in0=v[:, :n_t], scalar=NEG_SLOPE,
                        in1=v[:, :n_t], op0=ALU.mult, op1=ALU.max)
                    t2 = ew.tile([P, gch, H, C], bf16, tag="t2")
                    nc.vector.tensor_tensor(out=t2[:, :n_t], in0=e[:, :n_t],
                                            in1=attb_sb[:, :n_t], op=ALU.mult)
                    sc = ew.tile([P, gch, H, 1], f32, tag="sc")
                    nc.vector.tensor_reduce(out=sc[:, :n_t, :, 0],
                                            in_=t2[:, :n_t],
                                            axis=AX.X, op=ALU.add)
                    # 9. exp on the scalar engine, broadcast to full width so
                    # the msg multiply stays unit-stride (2x DVE mode)
                    exb = ew.tile([P, gch, H, C], bf16, tag="exb")
                    nc.scalar.activation(
                        exb[:, :n_t],
                        sc[:, :n_t].to_broadcast([P, n_t, H, C]), AF.Exp)
                    ex_t[k] = exb
                    # 10. msg = xg * ex (+ ones column for fused denominator)
                    if fuse_den:
                        msg = ew.tile([P, gch, 1, C + 1], bf16, tag="msg")
                        nc.vector.tensor_tensor(
                            out=msg[:, :n_t, :, :C], in0=xgr,
                            in1=exb[:, :n_t], op=ALU.mult)
                        nc.vector.tensor_copy(msg[:, :n_t, :, C],
                                              exb[:, :n_t, :, 0])
                        msg_t[k] = msg
                    else:
                        msg = ew.tile([P, gch, H, C], bf16, tag="msg")
                        nc.vector.tensor_tensor(
                            out=msg[:, :n_t], in0=xgr,
                            in1=exb[:, :n_t], op=ALU.mult)
                        msg_t[k] = msg

    nc.compile()
    return nc


# ---------------------------------------------------------------------------
# Host orchestration
# ---------------------------------------------------------------------------

def _rep(v, gch=1):
    """Replicate a 1-D param vector across 128 partitions (x gch copies)."""
    v = np.asarray(v, dtype=np.float32).reshape(-1)
    if gch > 1:
        v = np.tile(v, gch)
    return np.tile(v[None, :], (P, 1)).astype(BF16)


TRACE = False          # set by test harness to capture NTFF profiles
LAST_RESULTS = []      # BassKernelResults of the last kernel() call

GCH1 = 5               # edge-chunk tiles, layer 1 (PSUM: 1*5*2KB pxr)
GCH2 = 8               # layer 2 (F=128: 2*8*0.5KB pxr)
NQ = 4                 # SWDGE queues used by gathers (round-robin)


def run_spmd(nc, in_maps, trace=False, trace_kwargs=None):
    from concourse import bass_utils
    res = bass_utils.run_bass_kernel_spmd(
        nc, in_maps, core_ids=list(range(len(in_maps))), trace=trace or TRACE,
        **(trace_kwargs or {}))
    LAST_RESULTS.append(res)
    return res


def kernel(x, edge_index, W1l, b1l, W1r, b1r, att1, bias1,
           W2l, b2l, W2r, b2r, att2, bias2, Wa, ba, Wrc, brc):
    x = np.asarray(x, dtype=np.float32)
    Tb, per_core = prep_edges(edge_index)
    ones = np.ones((1, P), dtype=BF16)

    nc1 = _get_program(1, tuple(Tb))
    xT = np.ascontiguousarray(x.T).astype(BF16)
    common1 = {
        "xT": xT,
        "wl": np.asarray(W1l, np.float32).astype(BF16),
        "wr": np.asarray(W1r, np.float32).astype(BF16),
        "bl": np.asarray(b1l, np.float32).reshape(1, -1).astype(BF16),
        "br": np.asarray(b1r, np.float32).reshape(1, -1).astype(BF16),
        "attb": _rep(np.asarray(att1, np.float32).reshape(-1), GCH1),
        "biasb": _rep(bias1),
        "ones": ones,
    }
    in_maps = []
    for c in range(NCORES):
        m = dict(common1)
        m["xoT"] = np.ascontiguousarray(
            x[c * NPC:(c + 1) * NPC].T).astype(BF16)
        m["idx"] = per_core[c]["idx"]
        m["oh"] = per_core[c]["oh"]
        m["ohT"] = per_core[c]["ohT"]
        in_maps.append(m)
    LAST_RESULTS.clear()
    res1 = run_spmd(nc1, in_maps)
    h = np.concatenate([np.asarray(res1.results[c]["h_own"])
                        for c in range(NCORES)], axis=0)

    nc2 = _get_program(2, tuple(Tb))
    hT = np.ascontiguousarray(h.T)  # already bf16
    common2 = {
        "xT": hT,
        "wl": np.asarray(W2l, np.float32).astype(BF16),
        "wr": np.asarray(W2r, np.float32).astype(BF16),
        "bl": np.asarray(b2l, np.float32).reshape(1, -1).astype(BF16),
        "br": np.asarray(b2r, np.float32).reshape(1, -1).astype(BF16),
        "attb": _rep(np.asarray(att2, np.float32).reshape(-1), GCH2),
        "biasb": _rep(bias2),
        "ones": ones,
        "headw": np.concatenate(
            [_rep(np.asarray(Wa, np.float32).reshape(-1)),
             _rep(np.asarray(Wrc, np.float32).reshape(-1))], axis=1),
        "headb": np.concatenate(
            [np.full((P, 1), np.float32(np.asarray(ba).reshape(())), np.float32),
             np.full((P, 1), np.float32(np.asarray(brc).reshape(())), np.float32)],
            axis=1),
    }
    in_maps2 = []
    for c in range(NCORES):
        m = dict(common2)
        m["xoT"] = np.ascontiguousarray(h[c * NPC:(c + 1) * NPC].T)
        m["idx"] = per_core[c]["idx"]
        m["oh"] = per_core[c]["oh"]
        m["ohT"] = per_core[c]["ohT"]
        in_maps2.append(m)
    res2 = run_spmd(nc2, in_maps2)
    an = np.concatenate([np.asarray(res2.results[c]["an"])
                         for c in range(NCORES)], axis=0)
    rc = np.concatenate([np.asarray(res2.results[c]["rc"])
                         for c in range(NCORES)], axis=0)
    # device returns pre-sigmoid logits; finish the heads here
    an = 1.0 / (1.0 + np.exp(-(an + np.float32(np.asarray(ba).reshape(())))))
    rc = 1.0 / (1.0 + np.exp(-(rc + np.float32(np.asarray(brc).reshape(())))))
    return an, rc


_PROGRAMS = {}


def _get_program(layer, tb_key):
    key = (layer, tb_key)
    if key not in _PROGRAMS:
        if layer == 1:
            _PROGRAMS[key] = build_layer(N_NODES, NPC, IN_DIM, HEADS,
                                         list(tb_key), elu=True,
                                         heads_out=False, gch=GCH1)
        else:
            _PROGRAMS[key] = build_layer(N_NODES, NPC, HEADS * HID, 1,
                                         list(tb_key), elu=False,
                                         heads_out=True, gch=GCH2)
    return _PROGRAMS[key]
